# revision 23
# baseline (speedup 1.0000x reference)
"""Decoder-only attention kernel for Trainium2 (8 NeuronCores).

Sharding: tensor-parallel over heads (4 groups of 4 heads) x data-parallel
over batch (2), = 8 cores. Each core computes, for its batch b and its 4
heads, the partial output  sum_h (softmax(causal(q_h k_h^T / 8)) v_h) @ O_h
as a [T, D] array; the host sums the 4 partials per batch and adds Ob.

On-core layout strategy ("transposed flash attention"):
  - host supplies xT = x[b].T  [D, T]; QK projections then produce
    qT/kT [dk, T] directly (dk on partitions).
  - scoresT[s, tq] = kT_chunk.T @ qT  (K=dk=64); the two heads of a pair
    sit at base partitions 0/64 so their K=64 matmuls land in disjoint
    PE row groups and overlap in hardware.
  - softmax denominator is accumulated free as a ones-column appended to
    v in the z matmul: zT_aug[65, tq] = [v | 1]^T @ expT  (K=s chunks)
  - v-bias is folded into the v projection as one extra K=1 matmul
    ([x | 1] @ [Wv ; vb]), matching reference semantics exactly.
  - zT [hdk, tq] is exactly the stationary layout the O projection needs.
  - causal: upper-triangle (s,tq) blocks skipped; diagonal blocks
    multiplied post-exp by one of 4 host-precomputed 0/1 masks.
All matmuls run as float32r (full PE rate); PSUM accumulates fp32.
"""

import sys

import numpy as np

if "/opt/trn_rl_repo" not in sys.path:
    sys.path.insert(0, "/opt/trn_rl_repo")

# Model shapes (hardcoded; kernel.py must be self-contained).
B = 2
T = 2048
D = 1024
NH = 16  # total heads
H = 4  # heads per core
DK = 64
NCORES = 8

TQ = 512  # tq chunk (matmul moving free dim)
SK = 128  # s chunk (partition dim of scoresT)

_PROGRAM_CACHE = {}

# test.py can flip these before calling kernel()
TRACE = False
TRACE_KWARGS = {}
USE_FP32R = True
MASK_ENGINE = "vector"  # or "gpsimd"
RB_MODE = "gpsimd"  # or "matmul"


def _build_program(t=T, d=D, reps=1):
    import concourse.bass as bass  # noqa: F401
    import concourse.mybir as mybir
    import concourse.tile as tile
    from concourse import bacc

    f32 = mybir.dt.float32
    mmdt = mybir.dt.float32r if USE_FP32R else mybir.dt.float32

    def r(ap):  # matmul-operand dtype view
        return ap.bitcast(mmdt) if ap.dtype != mmdt else ap

    P = 128
    DC = d // P  # d_model chunks (contraction for projections)
    SC = t // SK  # s chunks
    TC = t // TQ  # tq chunks
    VW = H * (DK + 1)  # v block width per s-chunk: 4 heads x (64 v + 1 ones)
    m = H * DK

    nc = bacc.Bacc(
        "TRN2",
        target_bir_lowering=False,
        debug=False,
        enable_asserts=True,
        num_devices=NCORES,
    )

    xT = nc.dram_tensor("xT", [d, t], mmdt, kind="ExternalInput").ap()
    wq = nc.dram_tensor("wq", [d, m], mmdt, kind="ExternalInput").ap()
    wk = nc.dram_tensor("wk", [d, m], mmdt, kind="ExternalInput").ap()
    wv = nc.dram_tensor("wv", [d, m], mmdt, kind="ExternalInput").ap()
    wo = nc.dram_tensor("wo", [m, d], mmdt, kind="ExternalInput").ap()
    qb = nc.dram_tensor("qb", [P, 2], f32, kind="ExternalInput").ap()
    kb = nc.dram_tensor("kb", [P, 2], f32, kind="ExternalInput").ap()
    vb = nc.dram_tensor("vb", [1, m], mmdt, kind="ExternalInput").ap()
    mask = nc.dram_tensor("mask", [P, P], f32, kind="ExternalInput").ap()
    out = nc.dram_tensor("out", [t, d], f32, kind="ExternalOutput").ap()

    with tile.TileContext(nc) as tc:
        from contextlib import ExitStack

        ctx = ExitStack()
        with ctx:
            const = ctx.enter_context(tc.tile_pool(name="const", bufs=1))

            # ---- resident SBUF tensors ----
            xT_sb = [
                const.tile([P, t], mmdt, tag=f"xT{c}", name=f"xT{c}")
                for c in range(DC)
            ]
            wq_sb = const.tile([P, DC * m], mmdt, tag="wq")
            wk_sb = const.tile([P, DC * m], mmdt, tag="wk")
            wv_sb = const.tile([P, DC * m], mmdt, tag="wv")
            wo_sb = const.tile([P, 2 * d], mmdt, tag="wo")
            qb_sb = const.tile([P, 2], f32, tag="qb")
            kb_sb = const.tile([P, 2], f32, tag="kb")
            vb_sb = const.tile([1, m], mmdt, tag="vb")
            mask_sb = const.tile([P, P], f32, tag="mask")
            ones_dk = const.tile([1, DK], mmdt, tag="ones_dk")
            ones_row = const.tile([1, P], mmdt, tag="ones_row")
            v_sb = const.tile([P, SC * VW], mmdt, tag="v")
            qT_sb = [
                const.tile([P, t], mmdt, tag=f"qT{p}", name=f"qT{p}") for p in range(2)
            ]
            kT_sb = [
                const.tile([P, t], mmdt, tag=f"kT{p}", name=f"kT{p}") for p in range(2)
            ]
            zT_sb = [
                const.tile([P, t], mmdt, tag=f"zT{p}", name=f"zT{p}") for p in range(2)
            ]

            # ---- input DMAs: weights/constants first, then xT streamed in
            # tp-major [128, TQ] pieces so projections start at ~4us
            nc.sync.dma_start(vb_sb[:], vb[:])
            for c in range(DC):
                nc.sync.dma_start(
                    wv_sb[:, c * m : (c + 1) * m], wv[c * P : (c + 1) * P, :]
                )
            for c in range(DC):
                nc.sync.dma_start(
                    xT_sb[c][:, 0:TQ], xT[c * P : (c + 1) * P, 0:TQ]
                )
            for c in range(DC):
                nc.sync.dma_start(
                    wq_sb[:, c * m : (c + 1) * m], wq[c * P : (c + 1) * P, :]
                )
                nc.sync.dma_start(
                    wk_sb[:, c * m : (c + 1) * m], wk[c * P : (c + 1) * P, :]
                )
            nc.sync.dma_start(qb_sb[:], qb[:])
            nc.sync.dma_start(kb_sb[:], kb[:])
            nc.sync.dma_start(mask_sb[:], mask[:])
            if TC > 1:
                for c in range(DC):
                    nc.sync.dma_start(
                        xT_sb[c][:, TQ : 2 * TQ],
                        xT[c * P : (c + 1) * P, TQ : 2 * TQ],
                    )
            for kc in range(2):
                nc.sync.dma_start(
                    wo_sb[:, kc * d : (kc + 1) * d], wo[kc * P : (kc + 1) * P, :]
                )
            for tp in range(2, TC):
                for c in range(DC):
                    nc.sync.dma_start(
                        xT_sb[c][:, tp * TQ : (tp + 1) * TQ],
                        xT[c * P : (c + 1) * P, tp * TQ : (tp + 1) * TQ],
                    )
            ones_stage = const.tile([P, 1], f32, tag="ones_stage")
            nc.vector.memset(ones_stage[:], 1.0)
            nc.vector.tensor_copy(ones_dk[:], ones_stage[0:1, :].to_broadcast([1, DK]))
            nc.vector.tensor_copy(ones_row[:], ones_stage[0:1, :].to_broadcast([1, P]))
            # ones columns of the v blocks (col 64 of each head's 65-wide slot)
            nc.vector.tensor_copy(
                v_sb.rearrange("p (j h c) -> p j h c", h=H, c=DK + 1)[:, :, :, DK :],
                ones_stage[:, None, None, :].to_broadcast([P, SC, H, 1]),
            )

            def _phases():
                mask_tt = (
                    nc.gpsimd.tensor_tensor
                    if MASK_ENGINE == "gpsimd"
                    else nc.vector.tensor_tensor
                )
                with tc.tile_pool(name="pproj", bufs=3, space="PSUM") as pproj:
                    for tp in range(TC):
                        sl = slice(tp * TQ, (tp + 1) * TQ)
                        # ---- v projection (+v bias) for this tp block ----
                        for i in range(4 * tp, 4 * tp + 4):
                            pv = pproj.tile([P, m], f32, tag="mm", name="pv")
                            for c in range(DC):
                                nc.tensor.matmul(
                                    pv[:],
                                    r(xT_sb[c][:, i * P : (i + 1) * P]),
                                    r(wv_sb[:, c * m : (c + 1) * m]),
                                    start=(c == 0),
                                    stop=False,
                                )
                            nc.tensor.matmul(
                                pv[:],
                                r(ones_row[0:1, :]),
                                r(vb_sb[0:1, :]),
                                start=False,
                                stop=True,
                            )
                            nc.vector.tensor_copy(
                                v_sb.rearrange("p (j h c) -> p j h c", h=H, c=DK + 1)[
                                    :, i, :, 0:DK
                                ],
                                pv.rearrange("p (h c) -> p h c", c=DK),
                            )
                        # ---- q/k projections for this tp block ----
                        for p in range(2):
                            pq = pproj.tile([P, TQ], f32, tag="mm", name="pq")
                            for c in range(DC):
                                nc.tensor.matmul(
                                    pq[:],
                                    r(wq_sb[:, c * m + p * P : c * m + (p + 1) * P]),
                                    r(xT_sb[c][:, sl]),
                                    start=(c == 0),
                                    stop=(c == DC - 1),
                                )
                            nc.vector.tensor_scalar_add(
                                qT_sb[p][:, sl], pq[:], qb_sb[:, p : p + 1]
                            )
                            pk = pproj.tile([P, TQ], f32, tag="mm", name="pk")
                            for c in range(DC):
                                nc.tensor.matmul(
                                    pk[:],
                                    r(wk_sb[:, c * m + p * P : c * m + (p + 1) * P]),
                                    r(xT_sb[c][:, sl]),
                                    start=(c == 0),
                                    stop=(c == DC - 1),
                                )
                            nc.vector.tensor_scalar_add(
                                kT_sb[p][:, sl], pk[:], kb_sb[:, p : p + 1]
                            )

                # ---- attention + O projection, tq-chunk outer ----
                # consecutive s-chunk pairs share one [128, 2*TQ] PSUM tile so
                # exp runs as a single wide ACT op
                with (
                    tc.tile_pool(name="pa", bufs=3, space="PSUM") as pa_pool,
                    tc.tile_pool(name="pz", bufs=2, space="PSUM") as pz_pool,
                    tc.tile_pool(name="expt", bufs=4) as exp_pool,
                    tc.tile_pool(name="rcp", bufs=2) as rcp_pool,
                    tc.tile_pool(name="rbs", bufs=2) as rbs_pool,
                    tc.tile_pool(name="osb", bufs=4) as out_pool,
                ):
                    def _o_proj(tq_c):
                        for i in range(4 * tq_c, 4 * tq_c + 4):
                            po = pa_pool.tile([P, d], f32, tag="pa", name="po")
                            for d2 in range(d // TQ):
                                for kc in range(2):
                                    nc.tensor.matmul(
                                        po[:, d2 * TQ : (d2 + 1) * TQ],
                                        r(zT_sb[kc][:, i * P : (i + 1) * P]),
                                        r(
                                            wo_sb[
                                                :,
                                                kc * d
                                                + d2 * TQ : kc * d
                                                + (d2 + 1) * TQ,
                                            ]
                                        ),
                                        start=(kc == 0),
                                        stop=(kc == 1),
                                    )
                            ot = out_pool.tile([P, d], f32, tag="osb", name="ot")
                            nc.vector.tensor_copy(ot[:], po[:])
                            nc.sync.dma_start(out[i * P : (i + 1) * P, :], ot[:])

                    for tcq in range(TC):
                        sl = slice(tcq * TQ, (tcq + 1) * TQ)
                        nsc = min(SC, 4 * tcq + 4)
                        for p in range(2):
                            pz = [
                                pz_pool.tile([DK + 1, TQ], f32, tag="pz", name="pz")
                                for _ in range(2)
                            ]
                            for jj in range(0, nsc, 2):
                                jpair = (jj, jj + 1)
                                pss, ets = [], []
                                # 4 scores MMs (2 heads x 2 s-chunks); each
                                # head pair at PE row groups 0-63 / 64-127
                                for hh in range(2):
                                    ps = pa_pool.tile(
                                        [P, 2 * TQ], f32, tag="pa", name="ps"
                                    )
                                    for u, j in enumerate(jpair):
                                        nc.tensor.matmul(
                                            ps[:, u * TQ : (u + 1) * TQ],
                                            r(
                                                kT_sb[p][
                                                    hh * DK : (hh + 1) * DK,
                                                    j * SK : (j + 1) * SK,
                                                ]
                                            ),
                                            r(qT_sb[p][hh * DK : (hh + 1) * DK, sl]),
                                            start=True,
                                            stop=True,
                                            skip_group_check=True,
                                        )
                                    pss.append(ps)
                                for hh in range(2):
                                    et = exp_pool.tile(
                                        [P, 2 * TQ], mmdt, tag="expt", name="et"
                                    )
                                    nc.scalar.activation(
                                        et[:],
                                        pss[hh][:],
                                        mybir.ActivationFunctionType.Exp,
                                        scale=0.125,
                                    )
                                    for u, j in enumerate(jpair):
                                        rdiag = j - 4 * tcq
                                        if rdiag >= 0:
                                            mask_tt(
                                                et[
                                                    :,
                                                    u * TQ
                                                    + rdiag * SK : u * TQ
                                                    + (rdiag + 1) * SK,
                                                ],
                                                et[
                                                    :,
                                                    u * TQ
                                                    + rdiag * SK : u * TQ
                                                    + (rdiag + 1) * SK,
                                                ],
                                                mask_sb[:],
                                                op=mybir.AluOpType.mult,
                                            )
                                    ets.append(et)
                                for u, j in enumerate(jpair):
                                    rdiag = j - 4 * tcq
                                    for hh in range(2):
                                        l = 2 * p + hh
                                        vap = r(
                                            v_sb[
                                                :,
                                                j * VW + l * (DK + 1) : j * VW
                                                + (l + 1) * (DK + 1),
                                            ]
                                        )
                                        eta = ets[hh]
                                        if rdiag < 0:
                                            nc.tensor.matmul(
                                                pz[hh][:],
                                                vap,
                                                r(eta[:, u * TQ : (u + 1) * TQ]),
                                                start=(j == 0),
                                                stop=False,
                                                skip_group_check=True,
                                            )
                                        else:
                                            nc.tensor.matmul(
                                                pz[hh][
                                                    :, rdiag * SK : (rdiag + 1) * SK
                                                ],
                                                vap,
                                                r(
                                                    eta[
                                                        :,
                                                        u * TQ
                                                        + rdiag * SK : u * TQ
                                                        + (rdiag + 1) * SK,
                                                    ]
                                                ),
                                                start=(j == 0),
                                                stop=True,
                                                skip_group_check=True,
                                            )
                                            if rdiag < 3:
                                                nc.tensor.matmul(
                                                    pz[hh][
                                                        :, (rdiag + 1) * SK : TQ
                                                    ],
                                                    vap,
                                                    r(
                                                        eta[
                                                            :,
                                                            u * TQ
                                                            + (rdiag + 1) * SK : u
                                                            * TQ
                                                            + TQ,
                                                        ]
                                                    ),
                                                    start=(j == 0),
                                                    stop=False,
                                                    skip_group_check=True,
                                                )
                            for hh in range(2):
                                # normalize: zT = zT_unnorm * (1/denom)
                                rcp = rcp_pool.tile(
                                    [1, TQ], mmdt, tag="rcp", name="rcp"
                                )
                                with nc.allow_low_precision(reason="fp32r recip"):
                                    nc.vector.reciprocal(
                                        rcp[:], pz[hh][DK : DK + 1, :]
                                    )
                                rb_sb = rbs_pool.tile(
                                    [DK, TQ], f32, tag="rbs", name="rbs"
                                )
                                nc.gpsimd.partition_broadcast(
                                    rb_sb[:], rcp.bitcast(f32)[:]
                                )
                                nc.vector.tensor_mul(
                                    zT_sb[p][hh * DK : (hh + 1) * DK, sl],
                                    pz[hh][0:DK, :],
                                    rb_sb[:],
                                )

                        # ---- O projection, pipelined one tq chunk behind ----
                        if tcq >= 1:
                            _o_proj(tcq - 1)
                    _o_proj(TC - 1)

            if reps == 1:
                _phases()
            else:
                with tc.For_i(0, reps, 1):
                    _phases()

    nc.compile()
    return nc


def _get_program(t=T, d=D, reps=1):
    key = (t, d, USE_FP32R, MASK_ENGINE, RB_MODE, reps)
    if key not in _PROGRAM_CACHE:
        _PROGRAM_CACHE[key] = _build_program(t, d, reps)
    return _PROGRAM_CACHE[key]


def _make_masks():
    # lower-triangular keep-mask for the exact diagonal 128x128 block
    i = np.arange(SK)[:, None]
    j = np.arange(SK)[None, :]
    return (i <= j).astype(np.float32)  # [128, 128]


def _core_inputs(x, Qs, Qbs, Ks, Kbs, Vs, Vbs, O, b, g, mask_host):
    hs = slice(H * g, H * (g + 1))
    xT_b = np.ascontiguousarray(x[b].T)  # [D, T]
    wq_g = np.ascontiguousarray(Qs[hs].transpose(1, 0, 2).reshape(D, H * DK))
    wk_g = np.ascontiguousarray(Ks[hs].transpose(1, 0, 2).reshape(D, H * DK))
    wv_g = np.ascontiguousarray(Vs[hs].transpose(1, 0, 2).reshape(D, H * DK))
    wo_g = np.ascontiguousarray(O[hs].reshape(H * DK, D))
    qb_flat = Qbs[hs].reshape(H * DK)
    kb_flat = Kbs[hs].reshape(H * DK)
    qb_g = np.ascontiguousarray(np.stack([qb_flat[0:128], qb_flat[128:256]], axis=1))
    kb_g = np.ascontiguousarray(np.stack([kb_flat[0:128], kb_flat[128:256]], axis=1))
    vb_g = np.ascontiguousarray(Vbs[hs].reshape(1, H * DK))
    return {
        "xT": xT_b,
        "wq": wq_g,
        "wk": wk_g,
        "wv": wv_g,
        "wo": wo_g,
        "qb": qb_g,
        "kb": kb_g,
        "vb": vb_g,
        "mask": mask_host,
    }


def _build_in_maps(x, Qs, Qbs, Ks, Kbs, Vs, Vbs, O_):
    mask_host = _make_masks()
    in_maps = []
    for core in range(NCORES):
        b, g = divmod(core, NH // H)
        in_maps.append(_core_inputs(x, Qs, Qbs, Ks, Kbs, Vs, Vbs, O_, b, g, mask_host))
    return in_maps


def benchmark(inputs, iters=20, warmup=3, reps=1):
    """Time repeated on-device executions with device-resident inputs.

    Returns (best_per_call_s, avg_per_call_s, burst_amortized_s).
    """
    import time

    import jax
    import numpy as _np
    from jax.experimental.shard_map import shard_map
    from jax.sharding import Mesh, PartitionSpec

    from concourse import bass2jax, mybir

    nc = _get_program(reps=reps)
    x = np.asarray(inputs["normalized_resid_pre"], np.float32)
    in_maps = _build_in_maps(
        x,
        np.asarray(inputs["Qs"], np.float32),
        np.asarray(inputs["Qbs"], np.float32),
        np.asarray(inputs["Ks"], np.float32),
        np.asarray(inputs["Kbs"], np.float32),
        np.asarray(inputs["Vs"], np.float32),
        np.asarray(inputs["Vbs"], np.float32),
        np.asarray(inputs["O"], np.float32),
    )

    bass2jax.install_neuronx_cc_hook()
    partition_name = nc.partition_id_tensor.name if nc.partition_id_tensor else None
    in_names, out_names, out_avals, zero_outs = [], [], [], []
    for alloc in nc.m.functions[0].allocations:
        if not isinstance(alloc, mybir.MemoryLocationSet):
            continue
        name = alloc.memorylocations[0].name
        if alloc.kind == "ExternalInput":
            if name != partition_name:
                in_names.append(name)
        elif alloc.kind == "ExternalOutput":
            out_names.append(name)
            dt = mybir.dt.np(alloc.dtype)
            out_avals.append(jax.core.ShapedArray(tuple(alloc.tensor_shape), dt))
            zero_outs.append(_np.zeros(tuple(alloc.tensor_shape), dt))
    n_params = len(in_names)
    all_names = in_names + out_names
    if partition_name is not None:
        all_names = all_names + [partition_name]

    def _body(*args):
        operands = list(args)
        if partition_name is not None:
            operands.append(bass2jax.partition_id_tensor())
        outs = bass2jax._bass_exec_p.bind(
            *operands,
            out_avals=tuple(out_avals),
            in_names=tuple(all_names),
            out_names=tuple(out_names),
            lowering_input_output_aliases=(),
            sim_require_finite=True,
            sim_require_nnan=True,
            nc=nc,
        )
        return tuple(outs)

    devices = jax.devices()[:NCORES]
    mesh = Mesh(_np.asarray(devices), ("core",))
    n_all = n_params + len(out_names)
    sharded = jax.jit(
        shard_map(
            _body,
            mesh=mesh,
            in_specs=(PartitionSpec("core"),) * n_all,
            out_specs=(PartitionSpec("core"),) * len(out_names),
            check_rep=False,
        ),
        keep_unused=True,
    )
    concat_in = [
        _np.concatenate([_np.asarray(in_maps[c][nm]) for c in range(NCORES)], axis=0)
        for nm in in_names
    ]
    sharding = jax.sharding.NamedSharding(mesh, PartitionSpec("core"))
    dev_in = [jax.device_put(a, sharding) for a in concat_in]
    dev_zeros = [
        jax.device_put(
            _np.zeros((NCORES * z.shape[0], *z.shape[1:]), z.dtype), sharding
        )
        for z in zero_outs
    ]

    for _ in range(warmup):
        jax.block_until_ready(sharded(*dev_in, *dev_zeros))
    times = []
    for _ in range(iters):
        t0 = time.perf_counter()
        jax.block_until_ready(sharded(*dev_in, *dev_zeros))
        times.append(time.perf_counter() - t0)
    t0 = time.perf_counter()
    rs = [sharded(*dev_in, *dev_zeros) for _ in range(iters)]
    jax.block_until_ready(rs)
    burst = (time.perf_counter() - t0) / iters
    return min(times), sum(times) / len(times), burst


def kernel(normalized_resid_pre, Qs, Qbs, Ks, Kbs, Vs, Vbs, O, Ob):
    from concourse.bass_utils import run_bass_kernel_spmd

    x = np.asarray(normalized_resid_pre, dtype=np.float32)
    Qs, Qbs = np.asarray(Qs, np.float32), np.asarray(Qbs, np.float32)
    Ks, Kbs = np.asarray(Ks, np.float32), np.asarray(Kbs, np.float32)
    Vs, Vbs = np.asarray(Vs, np.float32), np.asarray(Vbs, np.float32)
    O_, Ob = np.asarray(O, np.float32), np.asarray(Ob, np.float32)

    nc = _get_program()
    in_maps = _build_in_maps(x, Qs, Qbs, Ks, Kbs, Vs, Vbs, O_)

    res = run_bass_kernel_spmd(
        nc, in_maps, core_ids=list(range(NCORES)), trace=TRACE, **TRACE_KWARGS
    )
    kernel.last_results = res

    out = np.zeros((B, T, D), dtype=np.float32)
    for core in range(NCORES):
        b, g = divmod(core, NH // H)
        out[b] += res.results[core]["out"]
    out += Ob[None, None, :]
    return out


# revision 24
# speedup vs baseline: 1.0586x; 1.0586x over previous
"""Decoder-only attention kernel for Trainium2 (8 NeuronCores).

Sharding: tensor-parallel over heads (4 groups of 4 heads) x data-parallel
over batch (2), = 8 cores. Each core computes, for its batch b and its 4
heads, the partial output  sum_h (softmax(causal(q_h k_h^T / 8)) v_h) @ O_h
as a [T, D] array; the host sums the 4 partials per batch and adds Ob.

On-core layout strategy ("transposed flash attention"):
  - host supplies xT = x[b].T  [D, T]; QK projections then produce
    qT/kT [dk, T] directly (dk on partitions).
  - scoresT[s, tq] = kT_chunk.T @ qT  (K=dk=64); the two heads of a pair
    sit at base partitions 0/64 so their K=64 matmuls land in disjoint
    PE row groups and overlap in hardware.
  - softmax denominator is accumulated free as a ones-column appended to
    v in the z matmul: zT_aug[65, tq] = [v | 1]^T @ expT  (K=s chunks)
  - v-bias is folded into the v projection as one extra K=1 matmul
    ([x | 1] @ [Wv ; vb]), matching reference semantics exactly.
  - zT [hdk, tq] is exactly the stationary layout the O projection needs.
  - causal: upper-triangle (s,tq) blocks skipped; diagonal blocks
    multiplied post-exp by one of 4 host-precomputed 0/1 masks.
All matmuls run as float32r (full PE rate); PSUM accumulates fp32.
"""

import sys

import numpy as np

if "/opt/trn_rl_repo" not in sys.path:
    sys.path.insert(0, "/opt/trn_rl_repo")

# Model shapes (hardcoded; kernel.py must be self-contained).
B = 2
T = 2048
D = 1024
NH = 16  # total heads
H = 4  # heads per core
DK = 64
NCORES = 8

TQ = 512  # tq chunk (matmul moving free dim)
SK = 128  # s chunk (partition dim of scoresT)

_PROGRAM_CACHE = {}

# test.py can flip these before calling kernel()
TRACE = False
TRACE_KWARGS = {}
USE_FP32R = True
MASK_ENGINE = "vector"  # or "gpsimd"
RB_MODE = "matmul"  # or "gpsimd"


def _build_program(t=T, d=D, reps=1):
    import concourse.bass as bass  # noqa: F401
    import concourse.mybir as mybir
    import concourse.tile as tile
    from concourse import bacc

    f32 = mybir.dt.float32
    mmdt = mybir.dt.float32r if USE_FP32R else mybir.dt.float32

    def r(ap):  # matmul-operand dtype view
        return ap.bitcast(mmdt) if ap.dtype != mmdt else ap

    P = 128
    DC = d // P  # d_model chunks (contraction for projections)
    SC = t // SK  # s chunks
    TC = t // TQ  # tq chunks
    VW = H * (DK + 1)  # v block width per s-chunk: 4 heads x (64 v + 1 ones)
    m = H * DK

    nc = bacc.Bacc(
        "TRN2",
        target_bir_lowering=False,
        debug=False,
        enable_asserts=True,
        num_devices=NCORES,
    )

    xT = nc.dram_tensor("xT", [d, t], mmdt, kind="ExternalInput").ap()
    wq = nc.dram_tensor("wq", [d, m], mmdt, kind="ExternalInput").ap()
    wk = nc.dram_tensor("wk", [d, m], mmdt, kind="ExternalInput").ap()
    wv = nc.dram_tensor("wv", [d, m], mmdt, kind="ExternalInput").ap()
    wo = nc.dram_tensor("wo", [m, d], mmdt, kind="ExternalInput").ap()
    qb = nc.dram_tensor("qb", [P, 2], f32, kind="ExternalInput").ap()
    kb = nc.dram_tensor("kb", [P, 2], f32, kind="ExternalInput").ap()
    vb = nc.dram_tensor("vb", [1, m], mmdt, kind="ExternalInput").ap()
    mask = nc.dram_tensor("mask", [P, P], f32, kind="ExternalInput").ap()
    out = nc.dram_tensor("out", [t, d], f32, kind="ExternalOutput").ap()

    with tile.TileContext(nc) as tc:
        from contextlib import ExitStack

        ctx = ExitStack()
        with ctx:
            const = ctx.enter_context(tc.tile_pool(name="const", bufs=1))

            # ---- resident SBUF tensors ----
            xT_sb = [
                const.tile([P, t], mmdt, tag=f"xT{c}", name=f"xT{c}")
                for c in range(DC)
            ]
            wq_sb = const.tile([P, DC * m], mmdt, tag="wq")
            wk_sb = const.tile([P, DC * m], mmdt, tag="wk")
            wv_sb = const.tile([P, DC * m], mmdt, tag="wv")
            wo_sb = const.tile([P, 2 * d], mmdt, tag="wo")
            qb_sb = const.tile([P, 2], f32, tag="qb")
            kb_sb = const.tile([P, 2], f32, tag="kb")
            vb_sb = const.tile([1, m], mmdt, tag="vb")
            mask_sb = const.tile([P, P], f32, tag="mask")
            ones_dk = const.tile([1, DK], mmdt, tag="ones_dk")
            ones_row = const.tile([1, P], mmdt, tag="ones_row")
            v_sb = const.tile([P, SC * VW], mmdt, tag="v")
            qT_sb = [
                const.tile([P, t], mmdt, tag=f"qT{p}", name=f"qT{p}") for p in range(2)
            ]
            kT_sb = [
                const.tile([P, t], mmdt, tag=f"kT{p}", name=f"kT{p}") for p in range(2)
            ]
            zT_sb = [
                const.tile([P, t], mmdt, tag=f"zT{p}", name=f"zT{p}") for p in range(2)
            ]

            # ---- input DMAs: weights/constants first, then xT streamed in
            # tp-major [128, TQ] pieces so projections start at ~4us
            nc.sync.dma_start(vb_sb[:], vb[:])
            for c in range(DC):
                nc.sync.dma_start(
                    wv_sb[:, c * m : (c + 1) * m], wv[c * P : (c + 1) * P, :]
                )
            for c in range(DC):
                nc.sync.dma_start(
                    xT_sb[c][:, 0:TQ], xT[c * P : (c + 1) * P, 0:TQ]
                )
            for c in range(DC):
                nc.sync.dma_start(
                    wq_sb[:, c * m : (c + 1) * m], wq[c * P : (c + 1) * P, :]
                )
                nc.sync.dma_start(
                    wk_sb[:, c * m : (c + 1) * m], wk[c * P : (c + 1) * P, :]
                )
            nc.sync.dma_start(qb_sb[:], qb[:])
            nc.sync.dma_start(kb_sb[:], kb[:])
            nc.sync.dma_start(mask_sb[:], mask[:])
            if TC > 1:
                for c in range(DC):
                    nc.sync.dma_start(
                        xT_sb[c][:, TQ : 2 * TQ],
                        xT[c * P : (c + 1) * P, TQ : 2 * TQ],
                    )
            for kc in range(2):
                nc.sync.dma_start(
                    wo_sb[:, kc * d : (kc + 1) * d], wo[kc * P : (kc + 1) * P, :]
                )
            for tp in range(2, TC):
                for c in range(DC):
                    nc.sync.dma_start(
                        xT_sb[c][:, tp * TQ : (tp + 1) * TQ],
                        xT[c * P : (c + 1) * P, tp * TQ : (tp + 1) * TQ],
                    )
            ones_stage = const.tile([P, 1], f32, tag="ones_stage")
            nc.vector.memset(ones_stage[:], 1.0)
            nc.vector.tensor_copy(ones_dk[:], ones_stage[0:1, :].to_broadcast([1, DK]))
            nc.vector.tensor_copy(ones_row[:], ones_stage[0:1, :].to_broadcast([1, P]))
            # ones columns of the v blocks (col 64 of each head's 65-wide slot)
            nc.vector.tensor_copy(
                v_sb.rearrange("p (j h c) -> p j h c", h=H, c=DK + 1)[:, :, :, DK :],
                ones_stage[:, None, None, :].to_broadcast([P, SC, H, 1]),
            )

            def _phases():
                mask_tt = (
                    nc.gpsimd.tensor_tensor
                    if MASK_ENGINE == "gpsimd"
                    else nc.vector.tensor_tensor
                )
                with tc.tile_pool(name="pproj", bufs=3, space="PSUM") as pproj:
                    for tp in range(TC):
                        sl = slice(tp * TQ, (tp + 1) * TQ)
                        # ---- v projection (+v bias) for this tp block ----
                        for i in range(4 * tp, 4 * tp + 4):
                            pv = pproj.tile([P, m], f32, tag="mm", name="pv")
                            for c in range(DC):
                                nc.tensor.matmul(
                                    pv[:],
                                    r(xT_sb[c][:, i * P : (i + 1) * P]),
                                    r(wv_sb[:, c * m : (c + 1) * m]),
                                    start=(c == 0),
                                    stop=False,
                                )
                            nc.tensor.matmul(
                                pv[:],
                                r(ones_row[0:1, :]),
                                r(vb_sb[0:1, :]),
                                start=False,
                                stop=True,
                            )
                            nc.vector.tensor_copy(
                                v_sb.rearrange("p (j h c) -> p j h c", h=H, c=DK + 1)[
                                    :, i, :, 0:DK
                                ],
                                pv.rearrange("p (h c) -> p h c", c=DK),
                            )
                        # ---- q/k projections for this tp block ----
                        for p in range(2):
                            pq = pproj.tile([P, TQ], f32, tag="mm", name="pq")
                            for c in range(DC):
                                nc.tensor.matmul(
                                    pq[:],
                                    r(wq_sb[:, c * m + p * P : c * m + (p + 1) * P]),
                                    r(xT_sb[c][:, sl]),
                                    start=(c == 0),
                                    stop=(c == DC - 1),
                                )
                            nc.vector.tensor_scalar_add(
                                qT_sb[p][:, sl], pq[:], qb_sb[:, p : p + 1]
                            )
                            pk = pproj.tile([P, TQ], f32, tag="mm", name="pk")
                            for c in range(DC):
                                nc.tensor.matmul(
                                    pk[:],
                                    r(wk_sb[:, c * m + p * P : c * m + (p + 1) * P]),
                                    r(xT_sb[c][:, sl]),
                                    start=(c == 0),
                                    stop=(c == DC - 1),
                                )
                            nc.vector.tensor_scalar_add(
                                kT_sb[p][:, sl], pk[:], kb_sb[:, p : p + 1]
                            )

                # ---- attention + O projection, tq-chunk outer ----
                # consecutive s-chunk pairs share one [128, 2*TQ] PSUM tile so
                # exp runs as a single wide ACT op
                with (
                    tc.tile_pool(name="pa", bufs=2, space="PSUM") as pa_pool,
                    tc.tile_pool(name="pz", bufs=3, space="PSUM") as pz_pool,
                    tc.tile_pool(name="prb", bufs=1, space="PSUM") as rb_pool,
                    tc.tile_pool(name="expt", bufs=4) as exp_pool,
                    tc.tile_pool(name="rcp", bufs=2) as rcp_pool,
                    tc.tile_pool(name="rbs", bufs=2) as rbs_pool,
                    tc.tile_pool(name="osb", bufs=4) as out_pool,
                ):
                    def _o_proj(tq_c):
                        for i in range(4 * tq_c, 4 * tq_c + 4):
                            po = pa_pool.tile([P, d], f32, tag="pa", name="po")
                            for d2 in range(d // TQ):
                                for kc in range(2):
                                    nc.tensor.matmul(
                                        po[:, d2 * TQ : (d2 + 1) * TQ],
                                        r(zT_sb[kc][:, i * P : (i + 1) * P]),
                                        r(
                                            wo_sb[
                                                :,
                                                kc * d
                                                + d2 * TQ : kc * d
                                                + (d2 + 1) * TQ,
                                            ]
                                        ),
                                        start=(kc == 0),
                                        stop=(kc == 1),
                                    )
                            ot = out_pool.tile([P, d], f32, tag="osb", name="ot")
                            nc.vector.tensor_copy(ot[:], po[:])
                            nc.sync.dma_start(out[i * P : (i + 1) * P, :], ot[:])

                    for tcq in range(TC):
                        sl = slice(tcq * TQ, (tcq + 1) * TQ)
                        nsc = min(SC, 4 * tcq + 4)
                        for p in range(2):
                            pz = [
                                pz_pool.tile([DK + 1, TQ], f32, tag="pz", name="pz")
                                for _ in range(2)
                            ]
                            for jj in range(0, nsc, 2):
                                jpair = (jj, jj + 1)
                                pss, ets = [], []
                                # 4 scores MMs (2 heads x 2 s-chunks); each
                                # head pair at PE row groups 0-63 / 64-127
                                for hh in range(2):
                                    ps = pa_pool.tile(
                                        [P, 2 * TQ], f32, tag="pa", name="ps"
                                    )
                                    for u, j in enumerate(jpair):
                                        nc.tensor.matmul(
                                            ps[:, u * TQ : (u + 1) * TQ],
                                            r(
                                                kT_sb[p][
                                                    hh * DK : (hh + 1) * DK,
                                                    j * SK : (j + 1) * SK,
                                                ]
                                            ),
                                            r(qT_sb[p][hh * DK : (hh + 1) * DK, sl]),
                                            start=True,
                                            stop=True,
                                            skip_group_check=True,
                                        )
                                    pss.append(ps)
                                for hh in range(2):
                                    et = exp_pool.tile(
                                        [P, 2 * TQ], mmdt, tag="expt", name="et"
                                    )
                                    nc.scalar.activation(
                                        et[:],
                                        pss[hh][:],
                                        mybir.ActivationFunctionType.Exp,
                                        scale=0.125,
                                    )
                                    for u, j in enumerate(jpair):
                                        rdiag = j - 4 * tcq
                                        if rdiag >= 0:
                                            mask_tt(
                                                et[
                                                    :,
                                                    u * TQ
                                                    + rdiag * SK : u * TQ
                                                    + (rdiag + 1) * SK,
                                                ],
                                                et[
                                                    :,
                                                    u * TQ
                                                    + rdiag * SK : u * TQ
                                                    + (rdiag + 1) * SK,
                                                ],
                                                mask_sb[:],
                                                op=mybir.AluOpType.mult,
                                            )
                                    ets.append(et)
                                for u, j in enumerate(jpair):
                                    rdiag = j - 4 * tcq
                                    for hh in range(2):
                                        l = 2 * p + hh
                                        vap = r(
                                            v_sb[
                                                :,
                                                j * VW + l * (DK + 1) : j * VW
                                                + (l + 1) * (DK + 1),
                                            ]
                                        )
                                        eta = ets[hh]
                                        if rdiag < 0:
                                            nc.tensor.matmul(
                                                pz[hh][:],
                                                vap,
                                                r(eta[:, u * TQ : (u + 1) * TQ]),
                                                start=(j == 0),
                                                stop=False,
                                                skip_group_check=True,
                                            )
                                        else:
                                            nc.tensor.matmul(
                                                pz[hh][
                                                    :, rdiag * SK : (rdiag + 1) * SK
                                                ],
                                                vap,
                                                r(
                                                    eta[
                                                        :,
                                                        u * TQ
                                                        + rdiag * SK : u * TQ
                                                        + (rdiag + 1) * SK,
                                                    ]
                                                ),
                                                start=(j == 0),
                                                stop=True,
                                                skip_group_check=True,
                                            )
                                            if rdiag < 3:
                                                nc.tensor.matmul(
                                                    pz[hh][
                                                        :, (rdiag + 1) * SK : TQ
                                                    ],
                                                    vap,
                                                    r(
                                                        eta[
                                                            :,
                                                            u * TQ
                                                            + (rdiag + 1) * SK : u
                                                            * TQ
                                                            + TQ,
                                                        ]
                                                    ),
                                                    start=(j == 0),
                                                    stop=False,
                                                    skip_group_check=True,
                                                )
                            for hh in range(2):
                                # normalize: zT = zT_unnorm * (1/denom)
                                rcp = rcp_pool.tile(
                                    [1, TQ], mmdt, tag="rcp", name="rcp"
                                )
                                with nc.allow_low_precision(reason="fp32r recip"):
                                    nc.vector.reciprocal(
                                        rcp[:], pz[hh][DK : DK + 1, :]
                                    )
                                rb_sb = rbs_pool.tile(
                                    [DK, TQ], f32, tag="rbs", name="rbs"
                                )
                                if RB_MODE == "gpsimd":
                                    nc.gpsimd.partition_broadcast(
                                        rb_sb[:], rcp.bitcast(f32)[:]
                                    )
                                else:
                                    rb_ps = rb_pool.tile(
                                        [DK, TQ], f32, tag="rb", name="rb"
                                    )
                                    nc.tensor.matmul(
                                        rb_ps[:],
                                        r(ones_dk[0:1, :]),
                                        r(rcp[:]),
                                        start=True,
                                        stop=True,
                                    )
                                    nc.vector.tensor_copy(rb_sb[:], rb_ps[:])
                                nc.vector.tensor_mul(
                                    zT_sb[p][hh * DK : (hh + 1) * DK, sl],
                                    pz[hh][0:DK, :],
                                    rb_sb[:],
                                )

                        # ---- O projection, pipelined one tq chunk behind ----
                        if tcq >= 1:
                            _o_proj(tcq - 1)
                    _o_proj(TC - 1)

            if reps == 1:
                _phases()
            else:
                with tc.For_i(0, reps, 1):
                    _phases()

    nc.compile()
    return nc


def _get_program(t=T, d=D, reps=1):
    key = (t, d, USE_FP32R, MASK_ENGINE, RB_MODE, reps)
    if key not in _PROGRAM_CACHE:
        _PROGRAM_CACHE[key] = _build_program(t, d, reps)
    return _PROGRAM_CACHE[key]


def _make_masks():
    # lower-triangular keep-mask for the exact diagonal 128x128 block
    i = np.arange(SK)[:, None]
    j = np.arange(SK)[None, :]
    return (i <= j).astype(np.float32)  # [128, 128]


def _core_inputs(x, Qs, Qbs, Ks, Kbs, Vs, Vbs, O, b, g, mask_host):
    hs = slice(H * g, H * (g + 1))
    xT_b = np.ascontiguousarray(x[b].T)  # [D, T]
    wq_g = np.ascontiguousarray(Qs[hs].transpose(1, 0, 2).reshape(D, H * DK))
    wk_g = np.ascontiguousarray(Ks[hs].transpose(1, 0, 2).reshape(D, H * DK))
    wv_g = np.ascontiguousarray(Vs[hs].transpose(1, 0, 2).reshape(D, H * DK))
    wo_g = np.ascontiguousarray(O[hs].reshape(H * DK, D))
    qb_flat = Qbs[hs].reshape(H * DK)
    kb_flat = Kbs[hs].reshape(H * DK)
    qb_g = np.ascontiguousarray(np.stack([qb_flat[0:128], qb_flat[128:256]], axis=1))
    kb_g = np.ascontiguousarray(np.stack([kb_flat[0:128], kb_flat[128:256]], axis=1))
    vb_g = np.ascontiguousarray(Vbs[hs].reshape(1, H * DK))
    return {
        "xT": xT_b,
        "wq": wq_g,
        "wk": wk_g,
        "wv": wv_g,
        "wo": wo_g,
        "qb": qb_g,
        "kb": kb_g,
        "vb": vb_g,
        "mask": mask_host,
    }


def _build_in_maps(x, Qs, Qbs, Ks, Kbs, Vs, Vbs, O_):
    mask_host = _make_masks()
    in_maps = []
    for core in range(NCORES):
        b, g = divmod(core, NH // H)
        in_maps.append(_core_inputs(x, Qs, Qbs, Ks, Kbs, Vs, Vbs, O_, b, g, mask_host))
    return in_maps


def benchmark(inputs, iters=20, warmup=3, reps=1):
    """Time repeated on-device executions with device-resident inputs.

    Returns (best_per_call_s, avg_per_call_s, burst_amortized_s).
    """
    import time

    import jax
    import numpy as _np
    from jax.experimental.shard_map import shard_map
    from jax.sharding import Mesh, PartitionSpec

    from concourse import bass2jax, mybir

    nc = _get_program(reps=reps)
    x = np.asarray(inputs["normalized_resid_pre"], np.float32)
    in_maps = _build_in_maps(
        x,
        np.asarray(inputs["Qs"], np.float32),
        np.asarray(inputs["Qbs"], np.float32),
        np.asarray(inputs["Ks"], np.float32),
        np.asarray(inputs["Kbs"], np.float32),
        np.asarray(inputs["Vs"], np.float32),
        np.asarray(inputs["Vbs"], np.float32),
        np.asarray(inputs["O"], np.float32),
    )

    bass2jax.install_neuronx_cc_hook()
    partition_name = nc.partition_id_tensor.name if nc.partition_id_tensor else None
    in_names, out_names, out_avals, zero_outs = [], [], [], []
    for alloc in nc.m.functions[0].allocations:
        if not isinstance(alloc, mybir.MemoryLocationSet):
            continue
        name = alloc.memorylocations[0].name
        if alloc.kind == "ExternalInput":
            if name != partition_name:
                in_names.append(name)
        elif alloc.kind == "ExternalOutput":
            out_names.append(name)
            dt = mybir.dt.np(alloc.dtype)
            out_avals.append(jax.core.ShapedArray(tuple(alloc.tensor_shape), dt))
            zero_outs.append(_np.zeros(tuple(alloc.tensor_shape), dt))
    n_params = len(in_names)
    all_names = in_names + out_names
    if partition_name is not None:
        all_names = all_names + [partition_name]

    def _body(*args):
        operands = list(args)
        if partition_name is not None:
            operands.append(bass2jax.partition_id_tensor())
        outs = bass2jax._bass_exec_p.bind(
            *operands,
            out_avals=tuple(out_avals),
            in_names=tuple(all_names),
            out_names=tuple(out_names),
            lowering_input_output_aliases=(),
            sim_require_finite=True,
            sim_require_nnan=True,
            nc=nc,
        )
        return tuple(outs)

    devices = jax.devices()[:NCORES]
    mesh = Mesh(_np.asarray(devices), ("core",))
    n_all = n_params + len(out_names)
    sharded = jax.jit(
        shard_map(
            _body,
            mesh=mesh,
            in_specs=(PartitionSpec("core"),) * n_all,
            out_specs=(PartitionSpec("core"),) * len(out_names),
            check_rep=False,
        ),
        keep_unused=True,
    )
    concat_in = [
        _np.concatenate([_np.asarray(in_maps[c][nm]) for c in range(NCORES)], axis=0)
        for nm in in_names
    ]
    sharding = jax.sharding.NamedSharding(mesh, PartitionSpec("core"))
    dev_in = [jax.device_put(a, sharding) for a in concat_in]
    dev_zeros = [
        jax.device_put(
            _np.zeros((NCORES * z.shape[0], *z.shape[1:]), z.dtype), sharding
        )
        for z in zero_outs
    ]

    for _ in range(warmup):
        jax.block_until_ready(sharded(*dev_in, *dev_zeros))
    times = []
    for _ in range(iters):
        t0 = time.perf_counter()
        jax.block_until_ready(sharded(*dev_in, *dev_zeros))
        times.append(time.perf_counter() - t0)
    t0 = time.perf_counter()
    rs = [sharded(*dev_in, *dev_zeros) for _ in range(iters)]
    jax.block_until_ready(rs)
    burst = (time.perf_counter() - t0) / iters
    return min(times), sum(times) / len(times), burst


def kernel(normalized_resid_pre, Qs, Qbs, Ks, Kbs, Vs, Vbs, O, Ob):
    from concourse.bass_utils import run_bass_kernel_spmd

    x = np.asarray(normalized_resid_pre, dtype=np.float32)
    Qs, Qbs = np.asarray(Qs, np.float32), np.asarray(Qbs, np.float32)
    Ks, Kbs = np.asarray(Ks, np.float32), np.asarray(Kbs, np.float32)
    Vs, Vbs = np.asarray(Vs, np.float32), np.asarray(Vbs, np.float32)
    O_, Ob = np.asarray(O, np.float32), np.asarray(Ob, np.float32)

    nc = _get_program()
    in_maps = _build_in_maps(x, Qs, Qbs, Ks, Kbs, Vs, Vbs, O_)

    res = run_bass_kernel_spmd(
        nc, in_maps, core_ids=list(range(NCORES)), trace=TRACE, **TRACE_KWARGS
    )
    kernel.last_results = res

    out = np.zeros((B, T, D), dtype=np.float32)
    for core in range(NCORES):
        b, g = divmod(core, NH // H)
        out[b] += res.results[core]["out"]
    out += Ob[None, None, :]
    return out
